# revision 1
# baseline (speedup 1.0000x reference)
"""CombinedSparsity (spatial max-pool + lifetime top-k + max-unpool) on 8 TRN2 cores.

Strategy: shard the 128 channels across 8 cores (16 each). Per (b, c) map the
output is all zeros except (possibly) one element: the map's max, written back
at its argmax position, kept only if that max is among the top-6 over the batch
for its channel. So instead of writing a dense 268MB output, each core:
  1. streams its 33.5MB shard through one DVE max-reduce per channel-group,
  2. finds the per-channel top-8 batch entries with one InstMax/InstMaxIndex
     on the transposed pooled matrix,
  3. re-gathers only the 128 candidate maps (top-8 x 16 channels) via indirect
     DMA and locates each map's argmax by value-matching (InstMaxIndex),
  4. scatters the 96 surviving values (6 per channel) as single f32 elements
     with an offset-bounds-checked indirect DMA; everything else stays zero
     because PJRT output buffers are donated zero-filled.
"""
import numpy as np

import concourse.bass as bass
import concourse.bacc as bacc
import concourse.tile as tile
from concourse import mybir
from concourse.bass_utils import run_bass_kernel_spmd
from concourse.masks import make_identity

B = 128
C_FULL = 128
H = 64
W = 64
HW = H * W
N_CORES = 8
CSH = C_FULL // N_CORES      # channels per core
GROUP = 2                    # channels reduced per DVE instruction
N_GROUPS = CSH // GROUP
K = 6                        # lifetime top-k
F32 = mybir.dt.float32

_nc_cache = None


def _build():
    global _nc_cache
    if _nc_cache is not None:
        return _nc_cache

    nc = bacc.Bacc("TRN2", target_bir_lowering=False, debug=False)
    x = nc.dram_tensor("x", [B, CSH, HW], F32, kind="ExternalInput")
    y = nc.dram_tensor("y", [B, CSH, HW], F32, kind="ExternalOutput")
    x_flat = x.rearrange("b c h -> (b c) h")
    y_elem = y.rearrange("b c h -> (b c h)")[:, None]
    n_rows = B * CSH
    n_elem = B * CSH * HW
    BIG = float(2 * n_elem)  # 2^24, exact in f32

    # three channel units; each unit's top-k/gather/scatter tail is emitted
    # right after its loads/reduces so it overlaps the next unit's stream.
    # Small groups at the very start (pipeline spin-up) and very end (short
    # final serial chain).
    units = [
        (0, 16, [2, 2, 2, 2, 2, 2, 2, 1, 1]),
    ]

    with tile.TileContext(nc) as tc:
        with (
            tc.tile_pool(name="const", bufs=1) as cp,
            tc.tile_pool(name="gxp", bufs=5) as gxp,
            tc.tile_pool(name="small", bufs=1) as sp,
            tc.tile_pool(name="ps", bufs=1, space="PSUM") as pp,
        ):
            ident0 = cp.tile([B, B], F32)
            make_identity(nc, ident0[:])
            # keep matmul inputs single-producer-engine (DVE)
            ident = cp.tile([B, B], F32)
            nc.vector.tensor_copy(out=ident[:], in_=ident0[:])

            def emit_unit(u, c_lo, c_hi, groups):
                ncha = c_hi - c_lo
                nsurv = ncha * K

                # loads + per-(b,c) max over HW
                pooled = sp.tile([B, ncha], F32, name=f"pooled{u}")
                goff = 0
                for gsz in groups:
                    c0 = c_lo + goff
                    gx = gxp.tile([B, gsz * HW], F32, tag="gx")
                    nc.sync.dma_start(out=gx[:], in_=x[:, c0:c0 + gsz, :])
                    nc.vector.tensor_reduce(
                        out=pooled[:, goff:goff + gsz],
                        in_=gx[:].rearrange("p (c h) -> p c h", c=gsz),
                        axis=mybir.AxisListType.X,
                        op=mybir.AluOpType.max,
                    )
                    goff += gsz

                # per-channel top-8 over the batch
                pooled_t_ps = pp.tile([ncha, B], F32, name=f"ptps{u}")
                nc.tensor.transpose(
                    out=pooled_t_ps[:], in_=pooled[:], identity=ident[:]
                )
                pooled_t = sp.tile([ncha, B], F32, name=f"pt{u}")
                nc.vector.tensor_copy(out=pooled_t[:], in_=pooled_t_ps[:])

                pt8 = sp.tile([ncha, 8], F32, name=f"pt8{u}")
                nc.vector.max(out=pt8[:], in_=pooled_t[:])
                pi8 = sp.tile([ncha, 8], mybir.dt.uint32, name=f"pi8{u}")
                nc.vector.max_index(
                    out=pi8[:], in_max=pt8[:], in_values=pooled_t[:]
                )

                # r8[c, j] = b_idx*CSH + (c_lo+c): f32-exact DRAM row
                pi8f = sp.tile([ncha, 8], F32, name=f"pi8f{u}")
                nc.vector.tensor_copy(out=pi8f[:], in_=pi8[:])
                c_col_i = sp.tile([ncha, 1], mybir.dt.int32, name=f"cci{u}")
                nc.gpsimd.iota(
                    c_col_i[:], pattern=[[1, 1]], base=c_lo, channel_multiplier=1
                )
                c_col = sp.tile([ncha, 1], F32, name=f"cc{u}")
                nc.vector.tensor_copy(out=c_col[:], in_=c_col_i[:])
                r8 = sp.tile([ncha, 8], F32, name=f"r8{u}")
                nc.vector.tensor_scalar(
                    out=r8[:], in0=pi8f[:], scalar1=float(CSH),
                    scalar2=c_col[:, 0:1],
                    op0=mybir.AluOpType.mult, op1=mybir.AluOpType.add,
                )
                offp = sp.tile([ncha, 8], F32, name=f"offp{u}")
                nc.vector.tensor_scalar(
                    out=offp[:], in0=r8[:], scalar1=float(HW), scalar2=None,
                    op0=mybir.AluOpType.mult,
                )

                # compact the j<6 survivor slots: [ncha,6] -> [nsurv,1]
                compact_r_d = sp.tile([nsurv, 1], F32, name=f"crd{u}")
                nc.gpsimd.dma_start(out=compact_r_d[:], in_=r8[:, 0:K])
                compact_v_d = sp.tile([nsurv, 1], F32, name=f"cvd{u}")
                nc.gpsimd.dma_start(out=compact_v_d[:], in_=pt8[:, 0:K])
                compact_off_d = sp.tile([nsurv, 1], F32, name=f"cod{u}")
                nc.gpsimd.dma_start(out=compact_off_d[:], in_=offp[:, 0:K])

                compact_r_i = sp.tile([nsurv, 1], mybir.dt.int32, name=f"cri{u}")
                nc.vector.tensor_copy(out=compact_r_i[:], in_=compact_r_d[:])
                compact_v = sp.tile([nsurv, 1], F32, name=f"cv{u}")
                nc.vector.tensor_copy(out=compact_v[:], in_=compact_v_d[:])
                compact_off = sp.tile([nsurv, 1], F32, name=f"co{u}")
                nc.vector.tensor_copy(out=compact_off[:], in_=compact_off_d[:])

                # gather survivor maps
                cx = sp.tile([nsurv, HW], F32, name=f"cx{u}", tag="cx")
                nc.gpsimd.indirect_dma_start(
                    out=cx[:], out_offset=None,
                    in_=x_flat[:],
                    in_offset=bass.IndirectOffsetOnAxis(
                        ap=compact_r_i[:, 0:1], axis=0
                    ),
                )
                v8 = sp.tile([nsurv, 8], F32, name=f"v8{u}")
                nc.vector.tensor_copy(
                    out=v8[:], in_=compact_v[:, 0:1].to_broadcast([nsurv, 8])
                )
                hw8 = sp.tile([nsurv, 8], mybir.dt.uint32, name=f"hw8{u}")
                nc.vector.max_index(out=hw8[:], in_max=v8[:], in_values=cx[:])

                hwf = sp.tile([nsurv, 1], F32, name=f"hwf{u}")
                nc.vector.tensor_copy(out=hwf[:], in_=hw8[:, 0:1])
                coff_f = sp.tile([nsurv, 1], F32, name=f"cof{u}")
                nc.vector.tensor_tensor(
                    out=coff_f[:], in0=compact_off[:], in1=hwf[:],
                    op=mybir.AluOpType.add,
                )
                coff_i = sp.tile([nsurv, 1], mybir.dt.int32, name=f"coi{u}")
                nc.vector.tensor_copy(out=coff_i[:], in_=coff_f[:])

                nc.gpsimd.indirect_dma_start(
                    out=y_elem[:],
                    out_offset=bass.IndirectOffsetOnAxis(
                        ap=coff_i[:, 0:1], axis=0
                    ),
                    in_=compact_v[:],
                    in_offset=None,
                    bounds_check=n_elem - 1,
                    oob_is_err=False,
                )

            for u, (c_lo, c_hi, groups) in enumerate(units):
                emit_unit(u, c_lo, c_hi, groups)

    nc.finalize()
    _nc_cache = nc
    return nc


def _install_profile_hook():
    """Inject the antenv.axon_hooks shim so trace=True captures NTFFs."""
    import sys
    import types

    if "antenv.axon_hooks" in sys.modules:
        return
    import antenv
    import trn_agent_boot.trn_boot as tb

    mod = types.ModuleType("antenv.axon_hooks")
    mod._hook = tb._ntff_profile_via_ctypes("/opt/axon/libaxon_pjrt.so")
    mod.get_axon_ntff_profile_hook = lambda: mod._hook
    mod.set_axon_ntff_profile_hook = lambda h: setattr(mod, "_hook", h)
    sys.modules["antenv.axon_hooks"] = mod
    antenv.axon_hooks = mod

    # no S3 in this container — keep artifacts local
    import concourse.bass_utils as bu

    bu.upload_artifacts = lambda tmpdir: tmpdir


def run(activations, trace=False):
    if trace:
        _install_profile_hook()
    act = np.asarray(activations)
    assert act.shape == (B, C_FULL, H, W), act.shape
    act = act.astype(np.float32, copy=False)
    nc = _build()
    in_maps = [
        {"x": np.ascontiguousarray(act[:, i * CSH:(i + 1) * CSH]).reshape(B, CSH, HW)}
        for i in range(N_CORES)
    ]
    res = run_bass_kernel_spmd(
        nc, in_maps, core_ids=list(range(N_CORES)), trace=trace
    )
    out = np.concatenate(
        [r["y"].reshape(B, CSH, H, W) for r in res.results], axis=1
    )
    return out, res


def kernel(activations):
    out, _ = run(activations, trace=False)
    return out

